# revision 14
# baseline (speedup 1.0000x reference)
"""GCN (message-passing) Trainium2 Bass kernel, 8-core SPMD.

out = relu(scatter_add(norm * (x @ W_lin.T + b_lin)[src], dst) + x @ W_root.T + b_root)
with norm = dinv[src]*dinv[dst], dinv = rsqrt(max(in_degree, 1)).

Strategy (dst-sharding; the Bass program is specialized to the edge list at
build time, like the tile schedule):
  - Host: partition edges by dst owner core (6250 nodes/core), group by
    128-node dst block, pad each block segment to a multiple of 128 edges
    with a schedule shared across cores (max over cores). For each core lay
    the dinv[src]-prescaled bf16 source rows out in edge-slot order
    (partition-major: xe[p, t, :] = row of slot t*128+p), so the device
    reads them as a plain sequential stream — no per-edge descriptors.
  - Device, per core: stream xe chunks via HWDGE (16 KB/partition
    descriptors); per 128-edge tile build a one-hot S matrix
    (iota == dst_local) in bf16 with batched multi-tile DVE builds, and
    accumulate A^T[128, 64] = sum_tiles Xe^T S in PSUM on the PE (bf16,
    full-128-col stationary keeps the fast weight load on); per dst block
    scale by dinv[dst] (DVE, writes bf16), two small bf16 matmuls fold
    W_lin/W_root/b_lin/b_root; relu on the Scalar engine into one big
    bf16 SBUF tile, stored to HBM in two large DMAs (host casts to f32).
"""

import sys

import numpy as np

# concourse (Bass/Tile) lives in the container's trn_rl_repo checkout; make
# kernel.py importable from any working directory.
for _p in ("/opt/trn_rl_repo", "/root/.axon_site/_ro/trn_rl_repo"):
    if _p not in sys.path:
        sys.path.insert(0, _p)

N_CORES = 8
D = 96
ELEM = 128           # row width in the edge-slot stream (128 bf16 = 256 B);
                     # full 128-col stationary keeps the PE fast-weight-load on
BLK = 64             # dst nodes per block
CT = 64              # stream chunk size in 128-edge tiles (16 KB/partition)
SCT = 16             # S-build batch in tiles (per DVE instruction)
XG_BUFS = 4          # stream chunk buffers in flight



def _cdiv(a, b):
    return (a + b - 1) // b


def _bf16(a):
    import ml_dtypes

    return np.asarray(a, dtype=ml_dtypes.bfloat16)


def _prep(x, edge_index):
    """Host-side sharding/layout. Returns per-core input arrays + schedule."""
    N = x.shape[0]
    NPC = N // N_CORES
    NBLK = _cdiv(NPC, BLK)
    src = edge_index[0].astype(np.int64)
    dst = edge_index[1].astype(np.int64)

    deg = np.bincount(dst, minlength=N).astype(np.float32)
    dinv = (1.0 / np.sqrt(np.maximum(deg, 1.0))).astype(np.float32)
    w = np.zeros(N, np.float32)
    np.add.at(w, dst, dinv[src])
    c = (dinv * w).astype(np.float32)

    xs = np.zeros((N + 1, ELEM), np.float32)
    xs[:N, :D] = x * dinv[:, None]      # row N stays zero (pad slots)
    xs = _bf16(xs)

    # Degree-balanced dst relabeling: deal nodes (sorted by in-degree) cyclically
    # across the (core, block) bins so every block has ~equal edge count. This
    # equalizes the shared max-over-cores tile schedule, cutting pad slots.
    # perm[newpos] = orig node.
    nbins = N_CORES * NBLK
    cap = np.full(nbins, BLK, np.int64)
    cap[NBLK - 1::NBLK] = NPC - (NBLK - 1) * BLK
    order_nodes = np.argsort(-deg, kind="stable")
    perm = np.empty(N, np.int64)
    fill = np.zeros(nbins, np.int64)
    base = np.arange(N_CORES)[:, None] * NPC + np.arange(NBLK)[None, :] * BLK
    base = base.reshape(-1)
    bi = 0
    for nd in order_nodes:
        while fill[bi] >= cap[bi]:
            bi = (bi + 1) % nbins
        perm[base[bi] + fill[bi]] = nd
        fill[bi] += 1
        bi = (bi + 1) % nbins
    invp = np.empty(N, np.int64)
    invp[perm] = np.arange(N)
    dstn = invp[dst]

    cores = []
    counts = np.zeros((N_CORES, NBLK), np.int64)
    for cc in range(N_CORES):
        m = (dstn >= cc * NPC) & (dstn < (cc + 1) * NPC)
        s = src[m]
        dl = dstn[m] - cc * NPC
        blk = dl // BLK
        order = np.lexsort((s, blk))
        s, dl, blk = s[order], dl[order], blk[order]
        cores.append((s, dl))
        counts[cc] = np.bincount(blk, minlength=NBLK)

    # shared tile schedule: tiles per block = max over cores
    T = _cdiv(counts, 128).max(axis=0)          # [NBLK] (128-edge tiles)
    T = np.maximum(T, 1)  # paired-PSUM epilogue needs every block written
    seg_off = np.zeros(NBLK, np.int64)          # tile offset of each segment
    seg_off[1:] = np.cumsum(T)[:-1]
    t_total = int(T.sum())
    L = t_total * 128

    per_core = []
    for cc in range(N_CORES):
        s, dl = cores[cc]
        srcs = np.full(L, N, np.int64)            # pad slots read zero row N
        dloc_flat = np.full(L, -1.0, np.float32)  # pad slots give zero S rows
        pos = 0
        for b in range(NBLK):
            n = counts[cc, b]
            o = seg_off[b] * 128
            srcs[o:o + n] = s[pos:pos + n]
            dloc_flat[o:o + n] = (dl[pos:pos + n] - b * BLK).astype(np.float32)
            pos += n
        # edge-slot stream, partition-major: xe[p, t, :] = row of slot t*128+p
        xe = xs[srcs].reshape(t_total, 128, ELEM).transpose(1, 0, 2)
        xe = np.ascontiguousarray(xe).reshape(128, t_total * ELEM)
        dloc = _bf16(dloc_flat.reshape(t_total, 128).T)  # slot i -> [i%128, i//128]

        own = perm[cc * NPC:(cc + 1) * NPC]
        xroot = np.empty((D + 2, NPC), np.float32)
        xroot[:D] = x[own].T
        xroot[D] = 1.0
        xroot[D + 1] = c[own]
        dinvb = np.broadcast_to(dinv[own], (D, NPC))
        per_core.append({"xe": xe, "dloc": dloc,
                         "xroot": _bf16(xroot), "dinvb": _bf16(dinvb)})

    sched = {"N": N, "NPC": NPC, "NBLK": NBLK, "T": T, "seg_off": seg_off,
             "t_total": t_total, "L": L, "perm": perm}
    return per_core, sched


def _build(sched):
    import concourse.bacc as bacc
    import concourse.tile as tile
    from concourse import mybir, library_config

    NPC, NBLK = sched["NPC"], sched["NBLK"]
    T, seg_off, t_total = sched["T"], sched["seg_off"], sched["t_total"]

    f32 = mybir.dt.float32
    bf16 = mybir.dt.bfloat16
    eq, mult = mybir.AluOpType.is_equal, mybir.AluOpType.mult
    relu = mybir.ActivationFunctionType.Relu

    nc = bacc.Bacc("TRN2", target_bir_lowering=False, debug=False,
                   num_devices=N_CORES, num_swdge_queues=1,
                   dynamic_dma_scratch_size=8192)
    xe = nc.dram_tensor("xe", [128, t_total * ELEM], bf16,
                        kind="ExternalInput").ap()
    dloc = nc.dram_tensor("dloc", [128, t_total], bf16, kind="ExternalInput").ap()
    xroot = nc.dram_tensor("xroot", [D + 2, NPC], bf16, kind="ExternalInput").ap()
    dinvb = nc.dram_tensor("dinvb", [D, NPC], bf16, kind="ExternalInput").ap()
    wlin = nc.dram_tensor("wlin", [D, D], bf16, kind="ExternalInput").ap()
    wroot = nc.dram_tensor("wroot", [D + 2, D], bf16, kind="ExternalInput").ap()
    iota = nc.dram_tensor("iota", [128, BLK], bf16, kind="ExternalInput").ap()
    # output packed [128, NBLK//2, 96]: node (q*128+p) at out[p, q, :]
    outp = nc.dram_tensor("out", [128, (NBLK // 2) * D], bf16,
                          kind="ExternalOutput").ap()

    # chunk-size ramp: small chunks at both ends compress the startup
    # latency (first matmul waits on chunk 0) and the post-stream tail
    head, tail = [8, 16, 32], [32, 16, 8]
    rem = t_total - sum(head) - sum(tail)
    assert rem > 0
    mid = [CT] * (rem // CT)
    if rem % CT:
        mid.append(rem % CT)
    csizes = head + mid + tail
    cstart = [0]
    for c in csizes:
        cstart.append(cstart[-1] + c)
    chunk_of = []
    for ci, c in enumerate(csizes):
        chunk_of += [ci] * c

    with tile.TileContext(nc) as tc:
        with (
            tc.tile_pool(name="const", bufs=1) as cpool,
            tc.tile_pool(name="xg", bufs=XG_BUFS) as xg_pool,
            tc.tile_pool(name="s", bufs=10) as s_pool,
            tc.tile_pool(name="asb", bufs=4) as asb_pool,
            tc.tile_pool(name="psA", bufs=5, space="PSUM") as psA_pool,
            tc.tile_pool(name="psB", bufs=3, space="PSUM") as psB_pool,
        ):
            iota_t = cpool.tile([128, BLK], bf16)
            dloc_t = cpool.tile([128, t_total], bf16)
            xroot_t = cpool.tile([D + 2, NPC], bf16)
            dinvb_t = cpool.tile([D, NPC], bf16)
            wlin_t = cpool.tile([D, D], bf16)
            wroot_t = cpool.tile([D + 2, D], bf16)
            out_t = cpool.tile([128, (NBLK // 2) * D], bf16)
            nc.sync.dma_start(out=iota_t[:], in_=iota)
            nc.sync.dma_start(out=dloc_t[:], in_=dloc)

            # stream + one-hot chunk tiles (created lazily in stream order)
            chunks = []
            s_chunks = []

            def ensure_chunk(ci):
                while len(chunks) <= ci:
                    j = len(chunks)
                    t0 = cstart[j]
                    ct = csizes[j]
                    xt = xg_pool.tile([128, CT, ELEM], bf16, tag="xg")
                    eng = nc.sync if j % 2 == 0 else nc.scalar
                    eng.dma_start(out=xt[:, 0:ct, :],
                                  in_=xe[:, t0 * ELEM:(t0 + ct) * ELEM])
                    chunks.append(xt)
                return chunks[ci]

            def ensure_s_chunk(si):
                # one-hot rows for SCT tiles: S[p, t, j] =
                # (iota[p, j] == dloc[p, g0 + t])
                while len(s_chunks) <= si:
                    j = len(s_chunks)
                    g0 = j * SCT
                    st = min(SCT, t_total - g0)
                    S = s_pool.tile([128, SCT, BLK], bf16, tag="s")
                    nc.vector.tensor_tensor(
                        out=S[:, 0:st, :],
                        in0=iota_t[:].unsqueeze(1)
                            .to_broadcast([128, st, BLK]),
                        in1=dloc_t[:, g0:g0 + st].unsqueeze(2)
                            .to_broadcast([128, st, BLK]),
                        op=eq)
                    s_chunks.append(S)
                return s_chunks[si]

            # first stream chunks go ahead of the big epilogue tables on
            # both HWDGE queues (queues drain in program order; the PE
            # consumes tiles in order, so chunk 1 must not queue behind
            # 2.4 MB of tables)
            for ci in range(4):
                ensure_chunk(ci)
            nc.scalar.dma_start(out=wlin_t[:], in_=wlin)
            nc.scalar.dma_start(out=wroot_t[:], in_=wroot)
            h = NPC // 2
            nc.scalar.dma_start(out=xroot_t[:, 0:h], in_=xroot[:, 0:h])
            nc.scalar.dma_start(out=dinvb_t[:, 0:h], in_=dinvb[:, 0:h])
            nc.scalar.dma_start(out=xroot_t[:, h:], in_=xroot[:, h:])
            nc.scalar.dma_start(out=dinvb_t[:, h:], in_=dinvb[:, h:])

            NPAIR = NBLK // 2
            for q in range(NPAIR):
                bs = q * 2 * BLK
                rows2 = min(2 * BLK, NPC - bs)
                psA = psA_pool.tile([128, 2 * BLK], f32, name="psA", tag="psA")
                for half in (0, 1):
                    b = 2 * q + half
                    n_tiles = int(T[b])
                    for t in range(n_tiles):
                        g = int(seg_off[b]) + t         # global stream idx
                        xt = ensure_chunk(chunk_of[g])
                        S = ensure_s_chunk(g // SCT)
                        nc.tensor.matmul(
                            out=psA[:, half * BLK:(half + 1) * BLK],
                            lhsT=xt[:, g - cstart[chunk_of[g]], :],
                            rhs=S[:, g % SCT, :],
                            start=(t == 0), stop=(t == n_tiles - 1))

                asb = asb_pool.tile([D, 2 * BLK], bf16)
                nc.vector.tensor_tensor(
                    out=asb[:, 0:rows2], in0=psA[0:D, 0:rows2],
                    in1=dinvb_t[:, bs:bs + rows2], op=mult)
                psB = psB_pool.tile([2 * BLK, D], f32)
                nc.tensor.matmul(out=psB[0:rows2, :], lhsT=asb[:, 0:rows2],
                                 rhs=wlin_t[:], start=True, stop=False)
                nc.tensor.matmul(out=psB[0:rows2, :],
                                 lhsT=xroot_t[:, bs:bs + rows2],
                                 rhs=wroot_t[:], start=False, stop=True)
                nc.scalar.activation(out=out_t[0:rows2, q * D:(q + 1) * D],
                                     in_=psB[0:rows2, :], func=relu)
                if q == NPAIR // 2:
                    nc.sync.dma_start(out=outp[:, 0:(NPAIR // 2) * D],
                                      in_=out_t[:, 0:(NPAIR // 2) * D])
            nc.sync.dma_start(out=outp[:, (NPAIR // 2) * D:],
                              in_=out_t[:, (NPAIR // 2) * D:])

    nc.compile()
    return nc


def _make_inputs(inputs_np, per_core, sched):
    wlin_in = _bf16(inputs_np["W_lin"].T)
    wroot_in = np.empty((D + 2, D), np.float32)
    wroot_in[:D] = inputs_np["W_root"].T
    wroot_in[D] = inputs_np["b_root"]
    wroot_in[D + 1] = inputs_np["b_lin"]
    wroot_in = _bf16(wroot_in)
    iota_in = _bf16(np.broadcast_to(np.arange(BLK, dtype=np.float32),
                                    (128, BLK)))
    in_maps = []
    for cc in range(N_CORES):
        pc = per_core[cc]
        in_maps.append({
            "xe": pc["xe"], "dloc": pc["dloc"],
            "xroot": pc["xroot"], "dinvb": pc["dinvb"],
            "wlin": wlin_in, "wroot": wroot_in, "iota": iota_in,
        })
    return in_maps


def _unpack_out(res, sched):
    NPC, NBLK = sched["NPC"], sched["NBLK"]
    shards = []
    for cc in range(N_CORES):
        o = np.asarray(res.results[cc]["out"], np.float32)
        o = o.reshape(128, NBLK // 2, D)
        o = o.transpose(1, 0, 2).reshape((NBLK // 2) * 128, D)[:NPC]
        shards.append(o)
    shards = np.concatenate(shards, axis=0)
    out = np.empty_like(shards)
    out[sched["perm"]] = shards          # undo the dst relabeling
    return out


def kernel(x, edge_index, W_lin, b_lin, W_root, b_root):
    from concourse.bass_utils import run_bass_kernel_spmd

    x = np.asarray(x, dtype=np.float32)
    edge_index = np.asarray(edge_index)
    inputs_np = {"W_lin": np.asarray(W_lin, np.float32),
                 "b_lin": np.asarray(b_lin, np.float32),
                 "W_root": np.asarray(W_root, np.float32),
                 "b_root": np.asarray(b_root, np.float32)}

    per_core, sched = _prep(x, edge_index)
    nc = _build(sched)
    in_maps = _make_inputs(inputs_np, per_core, sched)
    res = run_bass_kernel_spmd(nc, in_maps, core_ids=list(range(N_CORES)))
    return _unpack_out(res, sched)


# revision 15
# speedup vs baseline: 1.1895x; 1.1895x over previous
"""GCN (message-passing) Trainium2 Bass kernel, 8-core SPMD.

out = relu(scatter_add(norm * (x @ W_lin.T + b_lin)[src], dst) + x @ W_root.T + b_root)
with norm = dinv[src]*dinv[dst], dinv = rsqrt(max(in_degree, 1)).

Strategy (dst-sharding; the Bass program is specialized to the edge list at
build time, like the tile schedule):
  - Host: partition edges by dst owner core (6250 nodes/core), group by
    128-node dst block, pad each block segment to a multiple of 128 edges
    with a schedule shared across cores (max over cores). For each core lay
    the dinv[src]-prescaled bf16 source rows out in edge-slot order
    (partition-major: xe[p, t, :] = row of slot t*128+p), so the device
    reads them as a plain sequential stream — no per-edge descriptors.
  - Device, per core: stream xe chunks via HWDGE (16 KB/partition
    descriptors); per 128-edge tile build a one-hot S matrix
    (iota == dst_local) in bf16 with batched multi-tile DVE builds, and
    accumulate A^T[128, 64] = sum_tiles Xe^T S in PSUM on the PE (bf16,
    full-128-col stationary keeps the fast weight load on); per dst block
    scale by dinv[dst] (DVE, writes bf16), two small bf16 matmuls fold
    W_lin/W_root/b_lin/b_root; relu on the Scalar engine into one big
    bf16 SBUF tile, stored to HBM in two large DMAs (host casts to f32).
"""

import sys

import numpy as np

# concourse (Bass/Tile) lives in the container's trn_rl_repo checkout; make
# kernel.py importable from any working directory.
for _p in ("/opt/trn_rl_repo", "/root/.axon_site/_ro/trn_rl_repo"):
    if _p not in sys.path:
        sys.path.insert(0, _p)

N_CORES = 8
D = 96
ELEM = 96            # packed row width in the edge-slot stream (96 bf16);
                     # the matmul reads an overlapping 128-col window so the
                     # PE fast-weight-load stays on — cols 96:128 are the next
                     # tile's data and only pollute unused psA rows 96:128
BLK = 64             # dst nodes per block
CT = 64              # stream chunk size in 128-edge tiles (16 KB/partition)
SCT = 16             # S-build batch in tiles (per DVE instruction)
XG_BUFS = 4          # stream chunk buffers in flight



def _cdiv(a, b):
    return (a + b - 1) // b


def _bf16(a):
    import ml_dtypes

    return np.asarray(a, dtype=ml_dtypes.bfloat16)


def _prep(x, edge_index):
    """Host-side sharding/layout. Returns per-core input arrays + schedule."""
    N = x.shape[0]
    NPC = N // N_CORES
    NBLK = _cdiv(NPC, BLK)
    src = edge_index[0].astype(np.int64)
    dst = edge_index[1].astype(np.int64)

    deg = np.bincount(dst, minlength=N).astype(np.float32)
    dinv = (1.0 / np.sqrt(np.maximum(deg, 1.0))).astype(np.float32)
    w = np.zeros(N, np.float32)
    np.add.at(w, dst, dinv[src])
    c = (dinv * w).astype(np.float32)

    xs = np.zeros((N + 1, D), np.float32)
    xs[:N] = x * dinv[:, None]          # row N stays zero (pad slots)
    xs = _bf16(xs)

    # Degree-balanced dst relabeling: deal nodes (sorted by in-degree) cyclically
    # across the (core, block) bins so every block has ~equal edge count. This
    # equalizes the shared max-over-cores tile schedule, cutting pad slots.
    # perm[newpos] = orig node.
    nbins = N_CORES * NBLK
    cap = np.full(nbins, BLK, np.int64)
    cap[NBLK - 1::NBLK] = NPC - (NBLK - 1) * BLK
    order_nodes = np.argsort(-deg, kind="stable")
    perm = np.empty(N, np.int64)
    fill = np.zeros(nbins, np.int64)
    base = np.arange(N_CORES)[:, None] * NPC + np.arange(NBLK)[None, :] * BLK
    base = base.reshape(-1)
    bi = 0
    for nd in order_nodes:
        while fill[bi] >= cap[bi]:
            bi = (bi + 1) % nbins
        perm[base[bi] + fill[bi]] = nd
        fill[bi] += 1
        bi = (bi + 1) % nbins
    invp = np.empty(N, np.int64)
    invp[perm] = np.arange(N)
    dstn = invp[dst]

    cores = []
    counts = np.zeros((N_CORES, NBLK), np.int64)
    for cc in range(N_CORES):
        m = (dstn >= cc * NPC) & (dstn < (cc + 1) * NPC)
        s = src[m]
        dl = dstn[m] - cc * NPC
        blk = dl // BLK
        order = np.lexsort((s, blk))
        s, dl, blk = s[order], dl[order], blk[order]
        cores.append((s, dl))
        counts[cc] = np.bincount(blk, minlength=NBLK)

    # shared tile schedule: tiles per block = max over cores
    T = _cdiv(counts, 128).max(axis=0)          # [NBLK] (128-edge tiles)
    T = np.maximum(T, 1)  # paired-PSUM epilogue needs every block written
    seg_off = np.zeros(NBLK, np.int64)          # tile offset of each segment
    seg_off[1:] = np.cumsum(T)[:-1]
    t_total = int(T.sum())
    L = t_total * 128

    per_core = []
    for cc in range(N_CORES):
        s, dl = cores[cc]
        srcs = np.full(L, N, np.int64)            # pad slots read zero row N
        dloc_flat = np.full(L, -1.0, np.float32)  # pad slots give zero S rows
        pos = 0
        for b in range(NBLK):
            n = counts[cc, b]
            o = seg_off[b] * 128
            srcs[o:o + n] = s[pos:pos + n]
            dloc_flat[o:o + n] = (dl[pos:pos + n] - b * BLK).astype(np.float32)
            pos += n
        # edge-slot stream, partition-major: xe[p, t*96:(t+1)*96] = row of
        # slot t*128+p, plus a 32-col tail pad for the overlapping window
        xe = xs[srcs].reshape(t_total, 128, ELEM).transpose(1, 0, 2)
        xe = np.ascontiguousarray(xe).reshape(128, t_total * ELEM)
        xe = np.concatenate([xe, np.zeros((128, 32), xe.dtype)], axis=1)
        dloc = _bf16(dloc_flat.reshape(t_total, 128).T)  # slot i -> [i%128, i//128]

        own = perm[cc * NPC:(cc + 1) * NPC]
        xroot = np.empty((D + 2, NPC), np.float32)
        xroot[:D] = x[own].T
        xroot[D] = 1.0
        xroot[D + 1] = c[own]
        dinvb = np.broadcast_to(dinv[own], (D, NPC))
        per_core.append({"xe": xe, "dloc": dloc,
                         "xroot": _bf16(xroot), "dinvb": _bf16(dinvb)})

    sched = {"N": N, "NPC": NPC, "NBLK": NBLK, "T": T, "seg_off": seg_off,
             "t_total": t_total, "L": L, "perm": perm}
    return per_core, sched


def _build(sched):
    import concourse.bacc as bacc
    import concourse.tile as tile
    from concourse import mybir, library_config

    NPC, NBLK = sched["NPC"], sched["NBLK"]
    T, seg_off, t_total = sched["T"], sched["seg_off"], sched["t_total"]

    f32 = mybir.dt.float32
    bf16 = mybir.dt.bfloat16
    eq, mult = mybir.AluOpType.is_equal, mybir.AluOpType.mult
    relu = mybir.ActivationFunctionType.Relu

    nc = bacc.Bacc("TRN2", target_bir_lowering=False, debug=False,
                   num_devices=N_CORES, num_swdge_queues=1,
                   dynamic_dma_scratch_size=8192)
    xe = nc.dram_tensor("xe", [128, t_total * ELEM + 32], bf16,
                        kind="ExternalInput").ap()
    dloc = nc.dram_tensor("dloc", [128, t_total], bf16, kind="ExternalInput").ap()
    xroot = nc.dram_tensor("xroot", [D + 2, NPC], bf16, kind="ExternalInput").ap()
    dinvb = nc.dram_tensor("dinvb", [D, NPC], bf16, kind="ExternalInput").ap()
    wlin = nc.dram_tensor("wlin", [D, D], bf16, kind="ExternalInput").ap()
    wroot = nc.dram_tensor("wroot", [D + 2, D], bf16, kind="ExternalInput").ap()
    iota = nc.dram_tensor("iota", [128, BLK], bf16, kind="ExternalInput").ap()
    # output packed [128, NBLK//2, 96]: node (q*128+p) at out[p, q, :]
    outp = nc.dram_tensor("out", [128, (NBLK // 2) * D], bf16,
                          kind="ExternalOutput").ap()

    # chunk-size ramp: small chunks at both ends compress the startup
    # latency (first matmul waits on chunk 0) and the post-stream tail
    head, tail = [8, 16, 32], [32, 16, 8]
    rem = t_total - sum(head) - sum(tail)
    assert rem > 0
    mid = [CT] * (rem // CT)
    if rem % CT:
        mid.append(rem % CT)
    csizes = head + mid + tail
    cstart = [0]
    for c in csizes:
        cstart.append(cstart[-1] + c)
    chunk_of = []
    for ci, c in enumerate(csizes):
        chunk_of += [ci] * c

    with tile.TileContext(nc) as tc:
        with (
            tc.tile_pool(name="const", bufs=1) as cpool,
            tc.tile_pool(name="xg", bufs=XG_BUFS) as xg_pool,
            tc.tile_pool(name="s", bufs=10) as s_pool,
            tc.tile_pool(name="asb", bufs=4) as asb_pool,
            tc.tile_pool(name="psA", bufs=5, space="PSUM") as psA_pool,
            tc.tile_pool(name="psB", bufs=3, space="PSUM") as psB_pool,
        ):
            iota_t = cpool.tile([128, BLK], bf16)
            dloc_t = cpool.tile([128, t_total], bf16)
            xroot_t = cpool.tile([D + 2, NPC], bf16)
            dinvb_t = cpool.tile([D, NPC], bf16)
            wlin_t = cpool.tile([D, D], bf16)
            wroot_t = cpool.tile([D + 2, D], bf16)
            out_t = cpool.tile([128, (NBLK // 2) * D], bf16)
            nc.sync.dma_start(out=iota_t[:], in_=iota)
            nc.sync.dma_start(out=dloc_t[:], in_=dloc)

            # stream + one-hot chunk tiles (created lazily in stream order)
            chunks = []
            s_chunks = []

            def ensure_chunk(ci):
                while len(chunks) <= ci:
                    j = len(chunks)
                    t0 = cstart[j]
                    ct = csizes[j]
                    xt = xg_pool.tile([128, CT * ELEM + 32], bf16, tag="xg")
                    eng = nc.sync if j % 2 == 0 else nc.scalar
                    eng.dma_start(
                        out=xt[:, 0:ct * ELEM + 32],
                        in_=xe[:, t0 * ELEM:(t0 + ct) * ELEM + 32])
                    chunks.append(xt)
                return chunks[ci]

            def ensure_s_chunk(si):
                # one-hot rows for SCT tiles: S[p, t, j] =
                # (iota[p, j] == dloc[p, g0 + t])
                while len(s_chunks) <= si:
                    j = len(s_chunks)
                    g0 = j * SCT
                    st = min(SCT, t_total - g0)
                    S = s_pool.tile([128, SCT, BLK], bf16, tag="s")
                    nc.vector.tensor_tensor(
                        out=S[:, 0:st, :],
                        in0=iota_t[:].unsqueeze(1)
                            .to_broadcast([128, st, BLK]),
                        in1=dloc_t[:, g0:g0 + st].unsqueeze(2)
                            .to_broadcast([128, st, BLK]),
                        op=eq)
                    s_chunks.append(S)
                return s_chunks[si]

            # first stream chunks go ahead of the big epilogue tables on
            # both HWDGE queues (queues drain in program order; the PE
            # consumes tiles in order, so chunk 1 must not queue behind
            # 2.4 MB of tables)
            for ci in range(4):
                ensure_chunk(ci)
            nc.scalar.dma_start(out=wlin_t[:], in_=wlin)
            nc.scalar.dma_start(out=wroot_t[:], in_=wroot)
            h = NPC // 2
            nc.scalar.dma_start(out=xroot_t[:, 0:h], in_=xroot[:, 0:h])
            nc.scalar.dma_start(out=dinvb_t[:, 0:h], in_=dinvb[:, 0:h])
            nc.scalar.dma_start(out=xroot_t[:, h:], in_=xroot[:, h:])
            nc.scalar.dma_start(out=dinvb_t[:, h:], in_=dinvb[:, h:])

            NPAIR = NBLK // 2
            for q in range(NPAIR):
                bs = q * 2 * BLK
                rows2 = min(2 * BLK, NPC - bs)
                psA = psA_pool.tile([128, 2 * BLK], f32, name="psA", tag="psA")
                for half in (0, 1):
                    b = 2 * q + half
                    n_tiles = int(T[b])
                    for t in range(n_tiles):
                        g = int(seg_off[b]) + t         # global stream idx
                        xt = ensure_chunk(chunk_of[g])
                        S = ensure_s_chunk(g // SCT)
                        nc.tensor.matmul(
                            out=psA[:, half * BLK:(half + 1) * BLK],
                            lhsT=xt[:, (g - cstart[chunk_of[g]]) * ELEM:
                                    (g - cstart[chunk_of[g]]) * ELEM + 128],
                            rhs=S[:, g % SCT, :],
                            start=(t == 0), stop=(t == n_tiles - 1))

                asb = asb_pool.tile([D, 2 * BLK], bf16)
                nc.vector.tensor_tensor(
                    out=asb[:, 0:rows2], in0=psA[0:D, 0:rows2],
                    in1=dinvb_t[:, bs:bs + rows2], op=mult)
                psB = psB_pool.tile([2 * BLK, D], f32)
                nc.tensor.matmul(out=psB[0:rows2, :], lhsT=asb[:, 0:rows2],
                                 rhs=wlin_t[:], start=True, stop=False)
                nc.tensor.matmul(out=psB[0:rows2, :],
                                 lhsT=xroot_t[:, bs:bs + rows2],
                                 rhs=wroot_t[:], start=False, stop=True)
                nc.scalar.activation(out=out_t[0:rows2, q * D:(q + 1) * D],
                                     in_=psB[0:rows2, :], func=relu)
                if q == NPAIR // 2:
                    nc.sync.dma_start(out=outp[:, 0:(NPAIR // 2) * D],
                                      in_=out_t[:, 0:(NPAIR // 2) * D])
            nc.sync.dma_start(out=outp[:, (NPAIR // 2) * D:],
                              in_=out_t[:, (NPAIR // 2) * D:])

    nc.compile()
    return nc


def _make_inputs(inputs_np, per_core, sched):
    wlin_in = _bf16(inputs_np["W_lin"].T)
    wroot_in = np.empty((D + 2, D), np.float32)
    wroot_in[:D] = inputs_np["W_root"].T
    wroot_in[D] = inputs_np["b_root"]
    wroot_in[D + 1] = inputs_np["b_lin"]
    wroot_in = _bf16(wroot_in)
    iota_in = _bf16(np.broadcast_to(np.arange(BLK, dtype=np.float32),
                                    (128, BLK)))
    in_maps = []
    for cc in range(N_CORES):
        pc = per_core[cc]
        in_maps.append({
            "xe": pc["xe"], "dloc": pc["dloc"],
            "xroot": pc["xroot"], "dinvb": pc["dinvb"],
            "wlin": wlin_in, "wroot": wroot_in, "iota": iota_in,
        })
    return in_maps


def _unpack_out(res, sched):
    NPC, NBLK = sched["NPC"], sched["NBLK"]
    shards = []
    for cc in range(N_CORES):
        o = np.asarray(res.results[cc]["out"], np.float32)
        o = o.reshape(128, NBLK // 2, D)
        o = o.transpose(1, 0, 2).reshape((NBLK // 2) * 128, D)[:NPC]
        shards.append(o)
    shards = np.concatenate(shards, axis=0)
    out = np.empty_like(shards)
    out[sched["perm"]] = shards          # undo the dst relabeling
    return out


def kernel(x, edge_index, W_lin, b_lin, W_root, b_root):
    from concourse.bass_utils import run_bass_kernel_spmd

    x = np.asarray(x, dtype=np.float32)
    edge_index = np.asarray(edge_index)
    inputs_np = {"W_lin": np.asarray(W_lin, np.float32),
                 "b_lin": np.asarray(b_lin, np.float32),
                 "W_root": np.asarray(W_root, np.float32),
                 "b_root": np.asarray(b_root, np.float32)}

    per_core, sched = _prep(x, edge_index)
    nc = _build(sched)
    in_maps = _make_inputs(inputs_np, per_core, sched)
    res = run_bass_kernel_spmd(nc, in_maps, core_ids=list(range(N_CORES)))
    return _unpack_out(res, sched)
